# revision 30
# baseline (speedup 1.0000x reference)
"""Capsule-network kernel for 8x TRN2 NeuronCores (data-parallel over batch).

Reference computation (see problem):
  prim = primary_input.reshape(B, 8, 1024)
  prev = zeros(B, 4096)
  for col in 0..3:
    # layer0: inp = [prim_t, x_t, col] (1537) @ W0 -> relu -> flat -> roll(-128)
    # layer1: inp = [x_t, col] (513) @ W1 -> relu -> flat -> roll(+128)
  out = prev @ W_out + b_out

Kernel strategy (per core, batch shard Bc=512):
  - Everything on-chip is FEATURE-MAJOR: tiles are [128 features, Bc batch].
    ROLL=128 == partition count, so rolls are free tile re-indexings.
  - The scalar `col` concat input contributes col*W[last_row] to the
    pre-activation -> folded into per-col biases (computed on host).
  - P = prim @ W0[0:1024] is col-invariant -> computed once (phase 1),
    kept in SBUF, added during the layer0 drain each col.
  - col 0 layer0 has x=0 -> out = relu(P + b0): no matmuls at all.
  - Matmuls run in bf16 (216ns b2b issue rate vs fp32r's 235ns; PSUM
    accumulation stays fp32, rel err ~4e-3 vs the 2e-2 gate).
  - PE p-state warmup: 4 dummy matmuls while the first DMAs are in
    flight (2KB-line packing made the DMA fast enough that more
    dummies would outlast it).
  - Cold-start DMA is 2-queue only (sync=prim, scalar=w0p then w1x):
    a third active queue costs prim a 1/3 fabric split.
  - L0 drains preload P into PSUM (start=False accumulation) so the
    psum->C latency is a single ACT op.

Perf ledger (per profile): ~260us tensor-busy (1184 matmuls, 216ns
issue), ~7us fixed framework preamble, ~4us semaphore-reset postamble,
~432ns instruction-fetch stall every ~49 matmuls (engine instruction
buffer refill; would need hardware loops to remove).
"""

import numpy as np

# ---- problem constants (hardcoded; kernel.py must be self-contained) ----
B_FULL = 4096
D_IN = 8192
T = 8            # NUM_TALL
NW = 4           # NUM_WIDE
F = 512          # feature size per capsule row
ROLL = 128
N_CORES = 8
BC = B_FULL // N_CORES   # per-core batch = 512
S = (F * T) // 128       # state feature tiles = 32
KP = (D_IN // T) // 128  # prim k-tiles per capsule row = 8
KX = F // 128            # x k-tiles = 4
FO = F // 128            # output feature tiles per row-layer = 4
N_OUT = 10

_CACHE = {}


def _build_program():
    """Build (and cache) the single-core Bass program. Same program runs
    SPMD on all 8 cores with different batch shards."""
    if "nc" in _CACHE:
        return _CACHE["nc"], _CACHE["names"]

    from contextlib import ExitStack

    import concourse.tile as tile
    from concourse import bacc, mybir

    f32 = mybir.dt.float32
    bf16 = mybir.dt.bfloat16
    AF = mybir.ActivationFunctionType
    ADD = mybir.AluOpType.add

    nc = bacc.Bacc("TRN2", target_bir_lowering=False, debug=False,
                   num_devices=N_CORES)

    # prim/w0p/w1x are packed on the host in k-tile PAIRS so every DMA
    # moves one contiguous 2KB line per partition (vs 2x1KB segments):
    # row g2*128+p holds [pair tile h=0 | h=1] side by side.
    prim_d = nc.dram_tensor("prim_t", [D_IN // 2, 2 * BC], bf16,
                            kind="ExternalInput").ap()
    w0p_d = nc.dram_tensor("w0p", [(KP // 2) * 128, 2 * F], bf16,
                           kind="ExternalInput").ap()
    w0x_d = nc.dram_tensor("w0x", [F, F], bf16, kind="ExternalInput").ap()
    w1x_d = nc.dram_tensor("w1x", [(KX // 2) * 128, 2 * F], bf16,
                           kind="ExternalInput").ap()
    wout_d = nc.dram_tensor("wout_packed", [128, S * N_OUT], bf16,
                            kind="ExternalInput").ap()
    bias0_d = nc.dram_tensor("bias0", [128, NW * FO], f32, kind="ExternalInput").ap()
    bias1_d = nc.dram_tensor("bias1", [128, NW * FO], f32, kind="ExternalInput").ap()
    bout_d = nc.dram_tensor("bout", [N_OUT, 1], f32, kind="ExternalInput").ap()
    out_d = nc.dram_tensor("out", [N_OUT, BC], f32, kind="ExternalOutput").ap()

    with tile.TileContext(nc) as tc, ExitStack() as ctx:
        const = ctx.enter_context(tc.tile_pool(name="const", bufs=1))
        state = ctx.enter_context(tc.tile_pool(name="state", bufs=1))
        cpool = ctx.enter_context(tc.tile_pool(name="cpool", bufs=10))
        prim_pool = ctx.enter_context(tc.tile_pool(name="primp", bufs=10))
        ppool = ctx.enter_context(tc.tile_pool(name="psum", bufs=8, space="PSUM"))

        # ---- constants (tiles only; DMAs are interleaved into phase 1 so
        # the first prim tiles hit SBUF as early as possible) ----
        # w0p packed as k-pairs [128, 2*F]: halves the DMA trigger count
        # (each DMA_DIRECT2D costs ~600ns of engine-queue issue time)
        w0p_sb = [const.tile([128, 2 * F], bf16, name=f"w0p{j}", tag=f"w0p{j}")
                  for j in range(KP // 2)]

        def w0p_ap(k, fo):
            return w0p_sb[k // 2][:, (k % 2) * F + fo * 128:
                                  (k % 2) * F + (fo + 1) * 128]
        w0x_sb = [const.tile([128, F], bf16, name=f"w0x{k}", tag=f"w0x{k}")
                  for k in range(KX)]
        w1x_sb = [const.tile([128, 2 * F], bf16, name=f"w1x{j}", tag=f"w1x{j}")
                  for j in range(KX // 2)]

        def w1x_ap(k, fo):
            return w1x_sb[k // 2][:, (k % 2) * F + fo * 128:
                                  (k % 2) * F + (fo + 1) * 128]
        wout_sb = const.tile([128, S * N_OUT], bf16, name="wout_sb", tag="wout")
        bias0_sb = const.tile([128, NW * FO], f32, name="bias0_sb", tag="bias0")
        bias1_sb = const.tile([128, NW * FO], f32, name="bias1_sb", tag="bias1")
        bout_sb = const.tile([N_OUT, 1], f32, name="bout_sb", tag="bout")

        def load_deferred_consts(t):
            # late-needed constants ride a separate DGE queue (scalar's) so
            # they never sit ahead of prim tiles in the sync queue
            if t == 0:
                nc.scalar.dma_start(bias0_sb[:], bias0_d[:, :])
            elif t == 1:
                # col0-L1 row 0 is EMITTED at t==1, so these DMAs must be
                # emitted no later (else read-before-write: the tile
                # framework only orders reads after writes that precede
                # them in program order). w1x rides the scalar queue
                # BEHIND w0p: a third active queue would cost prim a 1/3
                # fabric split; behind w0p it lands ~19us, just before
                # col0-L1 row 0 consumes it.
                nc.scalar.dma_start(bias1_sb[:], bias1_d[:, :])
                nc.scalar.dma_start(bout_sb[:], bout_d[:, :])
                for j in range(KX // 2):
                    nc.scalar.dma_start(w1x_sb[j][:],
                                        w1x_d[j * 128:(j + 1) * 128, :])
            elif t == 5:
                # deferred past the prim-stream peak (needed only from
                # col1-L0, ~90us in)
                for k in range(KX):
                    nc.scalar.dma_start(w0x_sb[k][:], w0x_d[k * 128:(k + 1) * 128, :])
            elif t == 7:
                nc.scalar.dma_start(wout_sb[:], wout_d[:, :])

        # ---- persistent state ----
        A = [state.tile([128, BC], bf16, name=f"state_a{i}", tag=f"A{i}")
             for i in range(S)]
        P = [state.tile([128, BC], f32, name=f"state_p{i}", tag=f"P{i}")
             for i in range(S)]

        def layer0_row(c, t, Cl):
            for fo in range(FO):
                j = t * FO + fo
                ct = cpool.tile([128, BC], bf16, name=f"c{c}_{j}", tag="C")
                b0ap = bias0_sb[:, c * FO + fo:c * FO + fo + 1]
                if c == 0:
                    # x == 0: out = relu(P + b0)
                    nc.scalar.activation(ct[:], P[j][:], AF.Relu, bias=b0ap)
                else:
                    ps = ppool.tile([128, BC], f32, name=f"ps0_{c}_{j}",
                                    tag="mm")
                    # preload P into the PSUM bank (DVE) and accumulate the
                    # matmuls on top (start=False): the drain is then a
                    # single ACT op, cutting ~750ns off the psum->C latency
                    # the L1 matmuls wait on once per row
                    nc.vector.tensor_copy(ps[:], P[j][:])
                    for k in range(KX):
                        x_ap = A[(t * FO + k - 1) % S]
                        nc.tensor.matmul(
                            ps[:],
                            w0x_sb[k][:, fo * 128:(fo + 1) * 128],
                            x_ap[:],
                            start=False, stop=(k == KX - 1),
                            skip_group_check=True)
                    nc.scalar.activation(ct[:], ps[:], AF.Relu, bias=b0ap)
                Cl[j] = ct

        def layer1_row(c, t, Cl):
            for fo in range(FO):
                j = t * FO + fo
                ps = ppool.tile([128, BC], f32, name=f"ps1_{c}_{j}", tag="mm")
                for k in range(KX):
                    x_ap = Cl[(t * FO + k + 1) % S]
                    nc.tensor.matmul(
                        ps[:],
                        w1x_ap(k, fo),
                        x_ap[:],
                        start=(k == 0), stop=(k == KX - 1))
                b1ap = bias1_sb[:, c * FO + fo:c * FO + fo + 1]
                if fo % 2 == 0:
                    nc.scalar.activation(A[j][:], ps[:], AF.Relu, bias=b1ap)
                else:
                    # relu(psum + bias) on DVE: (psum add bias) max 0
                    nc.vector.tensor_scalar(A[j][:], ps[:], b1ap, 0.0,
                                            ADD, mybir.AluOpType.max)

        # ---- PE p-state warmup ----
        # The PE clock ramps 0.65 -> 1.2 -> 2.4 GHz over ~3us of continuous
        # execution. While the first prim/w0p DMAs are in flight (the PE
        # would idle anyway), run dummy matmuls so the first real matmuls
        # issue at the full 216ns rate.
        warm = const.tile([128, BC], bf16, name="warm", tag="warm")
        nc.vector.memset(warm[:], 0.0)
        wps = ppool.tile([128, BC], f32, name="ps_warm", tag="mm")
        for i in range(4):
            nc.tensor.matmul(wps[:], warm[:, 0:128], warm[:],
                             start=(i == 0), stop=(i == 3))

        # ---- phase 1 fused with col 0 ----
        # P rows stream in per capsule row (k outer / fo inner so each prim
        # tile is read 4x back-to-back then released). col-0 layer0 is
        # ACT-only (x==0) and col-0 layer1's matmuls have no DMA dependency,
        # so interleaving them gives the PE work while prim streams in.
        Cl0 = [None] * S
        for t in range(T):
            pss = [ppool.tile([128, BC], f32, name=f"ps_p1_{t}_{fo}", tag="mm")
                   for fo in range(FO)]
            if t == 0:
                # w0p rides the scalar queue (4 paired triggers) so it
                # streams concurrently with prim on the sync queue
                for j in range(KP // 2):
                    nc.scalar.dma_start(w0p_sb[j][:],
                                        w0p_d[j * 128:(j + 1) * 128, :])
            # two k-tiles per DMA (3D access pattern) halves the DMA
            # trigger count on the sync queue
            for k2 in range(KP // 2):
                g = t * KP + 2 * k2
                pt = prim_pool.tile([128, 2 * BC], bf16,
                                    name=f"prim_{g}", tag="prim")
                g2 = t * (KP // 2) + k2
                nc.sync.dma_start(pt[:], prim_d[g2 * 128:(g2 + 1) * 128, :])
                for h in range(2):
                    k = 2 * k2 + h
                    for fo in range(FO):
                        nc.tensor.matmul(
                            pss[fo][:],
                            w0p_ap(k, fo),
                            pt[:, h * BC:(h + 1) * BC],
                            start=(k == 0), stop=(k == KP - 1))
            load_deferred_consts(t)
            for fo in range(FO):
                j = t * FO + fo
                nc.vector.tensor_copy(P[j][:], pss[fo][:])
                # col-0 layer0 (x==0): C = relu(psum + b0), read directly
                # from PSUM in parallel with the P copy
                ct = cpool.tile([128, BC], bf16, name=f"c0_{j}", tag="C")
                nc.scalar.activation(ct[:], pss[fo][:], AF.Relu,
                                     bias=bias0_sb[:, fo:fo + 1])
                Cl0[j] = ct
            if t >= 1:
                layer1_row(0, t - 1, Cl0)
        layer1_row(0, T - 1, Cl0)

        # ---- cols 1..3 of (layer0, layer1) ----
        # layer1 row t reads C tiles 4t+1..4t+4 (last one produced by layer0
        # row t+1), so emission interleaves: L0(s), L0(s+1), L1(s), L0(s+2),
        # L1(s+1), ..., L1(s+7). The start row rotates by one each col
        # (s = c) so the rows that depend on the previous col's last
        # layer1 writes are emitted last, leaving ~6 rows of pipeline slack
        # across each col boundary.
        for c in range(1, NW):
            Cl = [None] * S
            rows = [(c + i) % T for i in range(T)]
            layer0_row(c, rows[0], Cl)
            for i in range(1, T):
                layer0_row(c, rows[i], Cl)
                layer1_row(c, rows[i - 1], Cl)
            layer1_row(c, rows[T - 1], Cl)

        # ---- final: out = prev @ W_out + b_out;  prev[k] = A[(k-1) % S] ----
        psf_full = ppool.tile([128, BC], f32, name="psf", tag="mm")
        psf = psf_full[0:N_OUT, :]
        # emit in col-3's A-write order (rows 3..7,0..2 under the rotation)
        # so the accumulation chain never stalls on the tail of layer1
        ks = [(((3 + i // FO) % T) * FO + i % FO + 1) % S for i in range(S)]
        for i, k in enumerate(ks):
            nc.tensor.matmul(
                psf[:],
                wout_sb[:, k * N_OUT:(k + 1) * N_OUT],
                A[(k - 1) % S][:],
                start=(i == 0), stop=(i == S - 1))
        out_sb = cpool.tile([N_OUT, BC], f32, name="out_sb", tag="C")
        nc.scalar.activation(out_sb[:], psf[:], AF.Identity, bias=bout_sb[:])
        nc.sync.dma_start(out_d[:, :], out_sb[:])

    nc.compile()

    names = dict(prim="prim_t", w0p="w0p", w0x="w0x", w1x="w1x",
                 wout="wout_packed", bias0="bias0", bias1="bias1",
                 bout="bout", out="out")
    _CACHE["nc"] = nc
    _CACHE["names"] = names
    return nc, names


def _make_in_maps(primary_input, W0, b0, W1, b1, W_out, b_out):
    """Host-side sharding + layout prep (all cheap numpy except the
    feature-major transpose of the batch shards)."""
    import ml_dtypes

    bf16 = ml_dtypes.bfloat16
    primary_input = np.ascontiguousarray(primary_input, dtype=np.float32)
    W0 = np.asarray(W0, dtype=np.float32)
    b0 = np.asarray(b0, dtype=np.float32)
    W1 = np.asarray(W1, dtype=np.float32)
    b1 = np.asarray(b1, dtype=np.float32)
    W_out = np.asarray(W_out, dtype=np.float32)
    b_out = np.asarray(b_out, dtype=np.float32)

    def pack_pairs(m):
        # [n*256, c] -> [n*128, 2c]: row j*128+p holds [tile 2j | tile 2j+1]
        n = m.shape[0] // 256
        c = m.shape[1]
        return np.ascontiguousarray(
            m.reshape(n, 2, 128, c).transpose(0, 2, 1, 3).reshape(n * 128, 2 * c))

    ps = D_IN // T  # 1024
    w0p = pack_pairs(W0[:ps].astype(bf16))                   # [512, 1024]
    w0x = np.ascontiguousarray(W0[ps:ps + F].astype(bf16))   # [512, 512]
    w0_last = W0[ps + F]                             # [512]
    w1x = pack_pairs(W1[:F].astype(bf16))                    # [256, 1024]
    w1_last = W1[F]                                  # [512]

    bias0 = np.concatenate(
        [(b0 + c * w0_last).reshape(FO, 128).T for c in range(NW)], axis=1)
    bias1 = np.concatenate(
        [(b1 + c * w1_last).reshape(FO, 128).T for c in range(NW)], axis=1)
    bias0 = np.ascontiguousarray(bias0, dtype=np.float32)   # [128, 16]
    bias1 = np.ascontiguousarray(bias1, dtype=np.float32)   # [128, 16]

    # wout_packed[p, k*10+o] = W_out[128k+p, o]
    wout_packed = np.ascontiguousarray(
        W_out.reshape(S, 128, N_OUT).transpose(1, 0, 2)
        .reshape(128, S * N_OUT).astype(bf16))
    bout = np.ascontiguousarray(b_out.reshape(N_OUT, 1))

    shared = dict(w0p=w0p, w0x=w0x, w1x=w1x, wout_packed=wout_packed,
                  bias0=bias0, bias1=bias1, bout=bout)
    prim_bf = primary_input.astype(bf16)
    in_maps = []
    for core in range(N_CORES):
        shard = prim_bf[core * BC:(core + 1) * BC]                # [512, 8192]
        prim_t = pack_pairs(np.ascontiguousarray(shard.T))        # [4096, 1024]
        m = {"prim_t": prim_t}
        m.update(shared)
        in_maps.append(m)
    return in_maps


def _install_ntff_hook():
    """Provide antenv.axon_hooks (absent in this image) backed by ctypes
    calls into libaxon_pjrt.so, so run_bass_kernel_spmd(trace=True) can
    capture NTFF profiles. Mirrors trn_agent_boot.trn_boot."""
    import contextlib
    import ctypes
    import sys
    import types

    if "antenv.axon_hooks" in sys.modules:
        return
    so_path = "/opt/axon/libaxon_pjrt.so"
    lib = ctypes.CDLL(so_path)
    lib.axon_start_nrt_profile.argtypes = [ctypes.POINTER(ctypes.c_int64),
                                           ctypes.c_size_t]
    lib.axon_start_nrt_profile.restype = ctypes.c_int64
    lib.axon_stop_nrt_profile.argtypes = [ctypes.c_char_p]
    lib.axon_stop_nrt_profile.restype = ctypes.c_int64

    @contextlib.contextmanager
    def _hook(output_dir, device_ids):
        import jax
        jax.devices()
        if device_ids:
            ids = (ctypes.c_int64 * len(device_ids))(*device_ids)
            rc = lib.axon_start_nrt_profile(ids, len(device_ids))
        else:
            rc = lib.axon_start_nrt_profile(None, 0)
        if rc != 0:
            raise RuntimeError(f"axon_start_nrt_profile rc={rc}")
        try:
            yield
        finally:
            n = lib.axon_stop_nrt_profile(str(output_dir).encode())
            print(f"profile: {n} file(s) written to {output_dir}",
                  file=sys.stderr)

    mod = types.ModuleType("antenv.axon_hooks")
    mod.get_axon_ntff_profile_hook = lambda: _hook
    mod.set_axon_ntff_profile_hook = lambda h: None
    sys.modules["antenv.axon_hooks"] = mod
    import antenv
    antenv.axon_hooks = mod


def kernel(primary_input, W0, b0, W1, b1, W_out, b_out, _trace=False,
           _trace_cores=None):
    from concourse import bass_utils

    if _trace:
        _install_ntff_hook()

    nc, _ = _build_program()
    in_maps = _make_in_maps(primary_input, W0, b0, W1, b1, W_out, b_out)
    res = bass_utils.run_bass_kernel_spmd(
        nc, in_maps, core_ids=list(range(N_CORES)),
        trace=_trace, trace_cores=_trace_cores)
    out = np.empty((B_FULL, N_OUT), dtype=np.float32)
    for core in range(N_CORES):
        out[core * BC:(core + 1) * BC] = res.results[core]["out"].T
    if _trace:
        kernel._last_results = res
    return out



# revision 31
# speedup vs baseline: 1.0066x; 1.0066x over previous
"""Capsule-network kernel for 8x TRN2 NeuronCores (data-parallel over batch).

Reference computation (see problem):
  prim = primary_input.reshape(B, 8, 1024)
  prev = zeros(B, 4096)
  for col in 0..3:
    # layer0: inp = [prim_t, x_t, col] (1537) @ W0 -> relu -> flat -> roll(-128)
    # layer1: inp = [x_t, col] (513) @ W1 -> relu -> flat -> roll(+128)
  out = prev @ W_out + b_out

Kernel strategy (per core, batch shard Bc=512):
  - Everything on-chip is FEATURE-MAJOR: tiles are [128 features, Bc batch].
    ROLL=128 == partition count, so rolls are free tile re-indexings.
  - The scalar `col` concat input contributes col*W[last_row] to the
    pre-activation -> folded into per-col biases (computed on host).
  - P = prim @ W0[0:1024] is col-invariant -> computed once (phase 1),
    kept in SBUF, added during the layer0 drain each col.
  - col 0 layer0 has x=0 -> out = relu(P + b0): no matmuls at all.
  - Matmuls run in bf16 (216ns b2b issue rate vs fp32r's 235ns; PSUM
    accumulation stays fp32, rel err ~4e-3 vs the 2e-2 gate).
  - PE p-state warmup: 9 dummy matmuls while the first DMAs are in
    flight, so the clock ramp (0.65->2.4GHz over ~3us) completes on
    throwaway work and real matmuls issue at the full 216ns rate.
    (Fewer dummies measured worse: the ramp penalty shifts onto real
    matmuls and DMA-start jitter adds data stalls.)
  - Cold-start DMA is 2-queue only (sync=prim, scalar=w0p then w1x):
    a third active queue costs prim a 1/3 fabric split.
  - L0 drains preload P into PSUM (start=False accumulation) so the
    psum->C latency is a single ACT op.

Perf ledger (per profile): ~260us tensor-busy (1184 matmuls, 216ns
issue), ~7us fixed framework preamble, ~4us semaphore-reset postamble,
~432ns instruction-fetch stall every ~49 matmuls (engine instruction
buffer refill; would need hardware loops to remove).
"""

import numpy as np

# ---- problem constants (hardcoded; kernel.py must be self-contained) ----
B_FULL = 4096
D_IN = 8192
T = 8            # NUM_TALL
NW = 4           # NUM_WIDE
F = 512          # feature size per capsule row
ROLL = 128
N_CORES = 8
BC = B_FULL // N_CORES   # per-core batch = 512
S = (F * T) // 128       # state feature tiles = 32
KP = (D_IN // T) // 128  # prim k-tiles per capsule row = 8
KX = F // 128            # x k-tiles = 4
FO = F // 128            # output feature tiles per row-layer = 4
N_OUT = 10

_CACHE = {}


def _build_program():
    """Build (and cache) the single-core Bass program. Same program runs
    SPMD on all 8 cores with different batch shards."""
    if "nc" in _CACHE:
        return _CACHE["nc"], _CACHE["names"]

    from contextlib import ExitStack

    import concourse.tile as tile
    from concourse import bacc, mybir

    f32 = mybir.dt.float32
    bf16 = mybir.dt.bfloat16
    AF = mybir.ActivationFunctionType
    ADD = mybir.AluOpType.add

    nc = bacc.Bacc("TRN2", target_bir_lowering=False, debug=False,
                   num_devices=N_CORES)

    # prim/w0p/w1x are packed on the host in k-tile PAIRS so every DMA
    # moves one contiguous 2KB line per partition (vs 2x1KB segments):
    # row g2*128+p holds [pair tile h=0 | h=1] side by side.
    prim_d = nc.dram_tensor("prim_t", [D_IN // 2, 2 * BC], bf16,
                            kind="ExternalInput").ap()
    w0p_d = nc.dram_tensor("w0p", [(KP // 2) * 128, 2 * F], bf16,
                           kind="ExternalInput").ap()
    w0x_d = nc.dram_tensor("w0x", [F, F], bf16, kind="ExternalInput").ap()
    w1x_d = nc.dram_tensor("w1x", [(KX // 2) * 128, 2 * F], bf16,
                           kind="ExternalInput").ap()
    wout_d = nc.dram_tensor("wout_packed", [128, S * N_OUT], bf16,
                            kind="ExternalInput").ap()
    bias0_d = nc.dram_tensor("bias0", [128, NW * FO], f32, kind="ExternalInput").ap()
    bias1_d = nc.dram_tensor("bias1", [128, NW * FO], f32, kind="ExternalInput").ap()
    bout_d = nc.dram_tensor("bout", [N_OUT, 1], f32, kind="ExternalInput").ap()
    out_d = nc.dram_tensor("out", [N_OUT, BC], f32, kind="ExternalOutput").ap()

    with tile.TileContext(nc) as tc, ExitStack() as ctx:
        const = ctx.enter_context(tc.tile_pool(name="const", bufs=1))
        state = ctx.enter_context(tc.tile_pool(name="state", bufs=1))
        cpool = ctx.enter_context(tc.tile_pool(name="cpool", bufs=10))
        prim_pool = ctx.enter_context(tc.tile_pool(name="primp", bufs=10))
        ppool = ctx.enter_context(tc.tile_pool(name="psum", bufs=8, space="PSUM"))

        # ---- constants (tiles only; DMAs are interleaved into phase 1 so
        # the first prim tiles hit SBUF as early as possible) ----
        # w0p packed as k-pairs [128, 2*F]: halves the DMA trigger count
        # (each DMA_DIRECT2D costs ~600ns of engine-queue issue time)
        w0p_sb = [const.tile([128, 2 * F], bf16, name=f"w0p{j}", tag=f"w0p{j}")
                  for j in range(KP // 2)]

        def w0p_ap(k, fo):
            return w0p_sb[k // 2][:, (k % 2) * F + fo * 128:
                                  (k % 2) * F + (fo + 1) * 128]
        w0x_sb = [const.tile([128, F], bf16, name=f"w0x{k}", tag=f"w0x{k}")
                  for k in range(KX)]
        w1x_sb = [const.tile([128, 2 * F], bf16, name=f"w1x{j}", tag=f"w1x{j}")
                  for j in range(KX // 2)]

        def w1x_ap(k, fo):
            return w1x_sb[k // 2][:, (k % 2) * F + fo * 128:
                                  (k % 2) * F + (fo + 1) * 128]
        wout_sb = const.tile([128, S * N_OUT], bf16, name="wout_sb", tag="wout")
        bias0_sb = const.tile([128, NW * FO], f32, name="bias0_sb", tag="bias0")
        bias1_sb = const.tile([128, NW * FO], f32, name="bias1_sb", tag="bias1")
        bout_sb = const.tile([N_OUT, 1], f32, name="bout_sb", tag="bout")

        def load_deferred_consts(t):
            # late-needed constants ride a separate DGE queue (scalar's) so
            # they never sit ahead of prim tiles in the sync queue
            if t == 0:
                nc.scalar.dma_start(bias0_sb[:], bias0_d[:, :])
            elif t == 1:
                # col0-L1 row 0 is EMITTED at t==1, so these DMAs must be
                # emitted no later (else read-before-write: the tile
                # framework only orders reads after writes that precede
                # them in program order). w1x rides the scalar queue
                # BEHIND w0p: a third active queue would cost prim a 1/3
                # fabric split; behind w0p it lands ~19us, just before
                # col0-L1 row 0 consumes it.
                nc.scalar.dma_start(bias1_sb[:], bias1_d[:, :])
                nc.scalar.dma_start(bout_sb[:], bout_d[:, :])
                for j in range(KX // 2):
                    nc.scalar.dma_start(w1x_sb[j][:],
                                        w1x_d[j * 128:(j + 1) * 128, :])
            elif t == 5:
                # deferred past the prim-stream peak (needed only from
                # col1-L0, ~90us in)
                for k in range(KX):
                    nc.scalar.dma_start(w0x_sb[k][:], w0x_d[k * 128:(k + 1) * 128, :])
            elif t == 7:
                nc.scalar.dma_start(wout_sb[:], wout_d[:, :])

        # ---- persistent state ----
        A = [state.tile([128, BC], bf16, name=f"state_a{i}", tag=f"A{i}")
             for i in range(S)]
        P = [state.tile([128, BC], f32, name=f"state_p{i}", tag=f"P{i}")
             for i in range(S)]

        def layer0_row(c, t, Cl):
            for fo in range(FO):
                j = t * FO + fo
                ct = cpool.tile([128, BC], bf16, name=f"c{c}_{j}", tag="C")
                b0ap = bias0_sb[:, c * FO + fo:c * FO + fo + 1]
                if c == 0:
                    # x == 0: out = relu(P + b0)
                    nc.scalar.activation(ct[:], P[j][:], AF.Relu, bias=b0ap)
                else:
                    ps = ppool.tile([128, BC], f32, name=f"ps0_{c}_{j}",
                                    tag="mm")
                    # preload P into the PSUM bank (DVE) and accumulate the
                    # matmuls on top (start=False): the drain is then a
                    # single ACT op, cutting ~750ns off the psum->C latency
                    # the L1 matmuls wait on once per row
                    nc.vector.tensor_copy(ps[:], P[j][:])
                    for k in range(KX):
                        x_ap = A[(t * FO + k - 1) % S]
                        nc.tensor.matmul(
                            ps[:],
                            w0x_sb[k][:, fo * 128:(fo + 1) * 128],
                            x_ap[:],
                            start=False, stop=(k == KX - 1),
                            skip_group_check=True)
                    nc.scalar.activation(ct[:], ps[:], AF.Relu, bias=b0ap)
                Cl[j] = ct

        def layer1_row(c, t, Cl):
            for fo in range(FO):
                j = t * FO + fo
                ps = ppool.tile([128, BC], f32, name=f"ps1_{c}_{j}", tag="mm")
                for k in range(KX):
                    x_ap = Cl[(t * FO + k + 1) % S]
                    nc.tensor.matmul(
                        ps[:],
                        w1x_ap(k, fo),
                        x_ap[:],
                        start=(k == 0), stop=(k == KX - 1))
                b1ap = bias1_sb[:, c * FO + fo:c * FO + fo + 1]
                if fo % 2 == 0:
                    nc.scalar.activation(A[j][:], ps[:], AF.Relu, bias=b1ap)
                else:
                    # relu(psum + bias) on DVE: (psum add bias) max 0
                    nc.vector.tensor_scalar(A[j][:], ps[:], b1ap, 0.0,
                                            ADD, mybir.AluOpType.max)

        # ---- PE p-state warmup ----
        # The PE clock ramps 0.65 -> 1.2 -> 2.4 GHz over ~3us of continuous
        # execution. While the first prim/w0p DMAs are in flight (the PE
        # would idle anyway), run dummy matmuls so the first real matmuls
        # issue at the full 216ns rate.
        warm = const.tile([128, BC], bf16, name="warm", tag="warm")
        nc.vector.memset(warm[:], 0.0)
        wps = ppool.tile([128, BC], f32, name="ps_warm", tag="mm")
        for i in range(9):
            nc.tensor.matmul(wps[:], warm[:, 0:128], warm[:],
                             start=(i == 0), stop=(i == 8))

        # ---- phase 1 fused with col 0 ----
        # P rows stream in per capsule row (k outer / fo inner so each prim
        # tile is read 4x back-to-back then released). col-0 layer0 is
        # ACT-only (x==0) and col-0 layer1's matmuls have no DMA dependency,
        # so interleaving them gives the PE work while prim streams in.
        Cl0 = [None] * S
        for t in range(T):
            pss = [ppool.tile([128, BC], f32, name=f"ps_p1_{t}_{fo}", tag="mm")
                   for fo in range(FO)]
            if t == 0:
                # w0p rides the scalar queue (4 paired triggers) so it
                # streams concurrently with prim on the sync queue
                for j in range(KP // 2):
                    nc.scalar.dma_start(w0p_sb[j][:],
                                        w0p_d[j * 128:(j + 1) * 128, :])
            # two k-tiles per DMA (3D access pattern) halves the DMA
            # trigger count on the sync queue
            for k2 in range(KP // 2):
                g = t * KP + 2 * k2
                pt = prim_pool.tile([128, 2 * BC], bf16,
                                    name=f"prim_{g}", tag="prim")
                g2 = t * (KP // 2) + k2
                nc.sync.dma_start(pt[:], prim_d[g2 * 128:(g2 + 1) * 128, :])
                for h in range(2):
                    k = 2 * k2 + h
                    for fo in range(FO):
                        nc.tensor.matmul(
                            pss[fo][:],
                            w0p_ap(k, fo),
                            pt[:, h * BC:(h + 1) * BC],
                            start=(k == 0), stop=(k == KP - 1))
            load_deferred_consts(t)
            for fo in range(FO):
                j = t * FO + fo
                nc.vector.tensor_copy(P[j][:], pss[fo][:])
                # col-0 layer0 (x==0): C = relu(psum + b0), read directly
                # from PSUM in parallel with the P copy
                ct = cpool.tile([128, BC], bf16, name=f"c0_{j}", tag="C")
                nc.scalar.activation(ct[:], pss[fo][:], AF.Relu,
                                     bias=bias0_sb[:, fo:fo + 1])
                Cl0[j] = ct
            if t >= 1:
                layer1_row(0, t - 1, Cl0)
        layer1_row(0, T - 1, Cl0)

        # ---- cols 1..3 of (layer0, layer1) ----
        # layer1 row t reads C tiles 4t+1..4t+4 (last one produced by layer0
        # row t+1), so emission interleaves: L0(s), L0(s+1), L1(s), L0(s+2),
        # L1(s+1), ..., L1(s+7). The start row rotates by one each col
        # (s = c) so the rows that depend on the previous col's last
        # layer1 writes are emitted last, leaving ~6 rows of pipeline slack
        # across each col boundary.
        for c in range(1, NW):
            Cl = [None] * S
            rows = [(c + i) % T for i in range(T)]
            layer0_row(c, rows[0], Cl)
            for i in range(1, T):
                layer0_row(c, rows[i], Cl)
                layer1_row(c, rows[i - 1], Cl)
            layer1_row(c, rows[T - 1], Cl)

        # ---- final: out = prev @ W_out + b_out;  prev[k] = A[(k-1) % S] ----
        psf_full = ppool.tile([128, BC], f32, name="psf", tag="mm")
        psf = psf_full[0:N_OUT, :]
        # emit in col-3's A-write order (rows 3..7,0..2 under the rotation)
        # so the accumulation chain never stalls on the tail of layer1
        ks = [(((3 + i // FO) % T) * FO + i % FO + 1) % S for i in range(S)]
        for i, k in enumerate(ks):
            nc.tensor.matmul(
                psf[:],
                wout_sb[:, k * N_OUT:(k + 1) * N_OUT],
                A[(k - 1) % S][:],
                start=(i == 0), stop=(i == S - 1))
        out_sb = cpool.tile([N_OUT, BC], f32, name="out_sb", tag="C")
        nc.scalar.activation(out_sb[:], psf[:], AF.Identity, bias=bout_sb[:])
        nc.sync.dma_start(out_d[:, :], out_sb[:])

    nc.compile()

    names = dict(prim="prim_t", w0p="w0p", w0x="w0x", w1x="w1x",
                 wout="wout_packed", bias0="bias0", bias1="bias1",
                 bout="bout", out="out")
    _CACHE["nc"] = nc
    _CACHE["names"] = names
    return nc, names


def _make_in_maps(primary_input, W0, b0, W1, b1, W_out, b_out):
    """Host-side sharding + layout prep (all cheap numpy except the
    feature-major transpose of the batch shards)."""
    import ml_dtypes

    bf16 = ml_dtypes.bfloat16
    primary_input = np.ascontiguousarray(primary_input, dtype=np.float32)
    W0 = np.asarray(W0, dtype=np.float32)
    b0 = np.asarray(b0, dtype=np.float32)
    W1 = np.asarray(W1, dtype=np.float32)
    b1 = np.asarray(b1, dtype=np.float32)
    W_out = np.asarray(W_out, dtype=np.float32)
    b_out = np.asarray(b_out, dtype=np.float32)

    def pack_pairs(m):
        # [n*256, c] -> [n*128, 2c]: row j*128+p holds [tile 2j | tile 2j+1]
        n = m.shape[0] // 256
        c = m.shape[1]
        return np.ascontiguousarray(
            m.reshape(n, 2, 128, c).transpose(0, 2, 1, 3).reshape(n * 128, 2 * c))

    ps = D_IN // T  # 1024
    w0p = pack_pairs(W0[:ps].astype(bf16))                   # [512, 1024]
    w0x = np.ascontiguousarray(W0[ps:ps + F].astype(bf16))   # [512, 512]
    w0_last = W0[ps + F]                             # [512]
    w1x = pack_pairs(W1[:F].astype(bf16))                    # [256, 1024]
    w1_last = W1[F]                                  # [512]

    bias0 = np.concatenate(
        [(b0 + c * w0_last).reshape(FO, 128).T for c in range(NW)], axis=1)
    bias1 = np.concatenate(
        [(b1 + c * w1_last).reshape(FO, 128).T for c in range(NW)], axis=1)
    bias0 = np.ascontiguousarray(bias0, dtype=np.float32)   # [128, 16]
    bias1 = np.ascontiguousarray(bias1, dtype=np.float32)   # [128, 16]

    # wout_packed[p, k*10+o] = W_out[128k+p, o]
    wout_packed = np.ascontiguousarray(
        W_out.reshape(S, 128, N_OUT).transpose(1, 0, 2)
        .reshape(128, S * N_OUT).astype(bf16))
    bout = np.ascontiguousarray(b_out.reshape(N_OUT, 1))

    shared = dict(w0p=w0p, w0x=w0x, w1x=w1x, wout_packed=wout_packed,
                  bias0=bias0, bias1=bias1, bout=bout)
    prim_bf = primary_input.astype(bf16)
    in_maps = []
    for core in range(N_CORES):
        shard = prim_bf[core * BC:(core + 1) * BC]                # [512, 8192]
        prim_t = pack_pairs(np.ascontiguousarray(shard.T))        # [4096, 1024]
        m = {"prim_t": prim_t}
        m.update(shared)
        in_maps.append(m)
    return in_maps


def _install_ntff_hook():
    """Provide antenv.axon_hooks (absent in this image) backed by ctypes
    calls into libaxon_pjrt.so, so run_bass_kernel_spmd(trace=True) can
    capture NTFF profiles. Mirrors trn_agent_boot.trn_boot."""
    import contextlib
    import ctypes
    import sys
    import types

    if "antenv.axon_hooks" in sys.modules:
        return
    so_path = "/opt/axon/libaxon_pjrt.so"
    lib = ctypes.CDLL(so_path)
    lib.axon_start_nrt_profile.argtypes = [ctypes.POINTER(ctypes.c_int64),
                                           ctypes.c_size_t]
    lib.axon_start_nrt_profile.restype = ctypes.c_int64
    lib.axon_stop_nrt_profile.argtypes = [ctypes.c_char_p]
    lib.axon_stop_nrt_profile.restype = ctypes.c_int64

    @contextlib.contextmanager
    def _hook(output_dir, device_ids):
        import jax
        jax.devices()
        if device_ids:
            ids = (ctypes.c_int64 * len(device_ids))(*device_ids)
            rc = lib.axon_start_nrt_profile(ids, len(device_ids))
        else:
            rc = lib.axon_start_nrt_profile(None, 0)
        if rc != 0:
            raise RuntimeError(f"axon_start_nrt_profile rc={rc}")
        try:
            yield
        finally:
            n = lib.axon_stop_nrt_profile(str(output_dir).encode())
            print(f"profile: {n} file(s) written to {output_dir}",
                  file=sys.stderr)

    mod = types.ModuleType("antenv.axon_hooks")
    mod.get_axon_ntff_profile_hook = lambda: _hook
    mod.set_axon_ntff_profile_hook = lambda h: None
    sys.modules["antenv.axon_hooks"] = mod
    import antenv
    antenv.axon_hooks = mod


def kernel(primary_input, W0, b0, W1, b1, W_out, b_out, _trace=False,
           _trace_cores=None):
    from concourse import bass_utils

    if _trace:
        _install_ntff_hook()

    nc, _ = _build_program()
    in_maps = _make_in_maps(primary_input, W0, b0, W1, b1, W_out, b_out)
    res = bass_utils.run_bass_kernel_spmd(
        nc, in_maps, core_ids=list(range(N_CORES)),
        trace=_trace, trace_cores=_trace_cores)
    out = np.empty((B_FULL, N_OUT), dtype=np.float32)
    for core in range(N_CORES):
        out[core * BC:(core + 1) * BC] = res.results[core]["out"].T
    if _trace:
        kernel._last_results = res
    return out

